# revision 1
# baseline (speedup 1.0000x reference)
"""APPNP GNN kernel for 8 Trainium2 NeuronCores.

h = 0.8 * D_in^{-1/2} A D_out^{-1/2} (X W^T + b) + 0.2 * (X W^T + b)

Strategy: dst-shard nodes across 8 cores. Each core computes h0 for its
own nodes (PE matmul), scales by rsqrt(out-degree), AllGathers the
bf16 table, then per-edge dma_gathers source rows from HBM and
segment-reduces them into per-dst-window PSUM tiles via one-hot
matmuls (S^T @ msgs). Degrees come from CSR rowptr diffs on-device.
"""

import os
import sys

sys.path.insert(0, "/opt/trn_rl_repo")

SKIP_EDGES = os.environ.get("K_SKIP_EDGES", "") == "1"
GATHER_ONLY = os.environ.get("K_GATHER_ONLY", "") == "1"

import numpy as np
import ml_dtypes

import concourse.bass as bass
import concourse.bacc as bacc
import concourse.tile as tile
import concourse.mybir as mybir
from concourse.bass_utils import run_bass_kernel_spmd

F32 = mybir.dt.float32
BF16 = mybir.dt.bfloat16
I16 = mybir.dt.int16
I32 = mybir.dt.int32

NCORES = 8
SLAB_TILES = 32  # 4096 edges per dma_gather, rotated over 4 SWDGE queues
ALPHA = 0.2
TPB = 4  # node tiles per phase-1 matmul group


def _cfg(N, F, C):
    sh = (N + NCORES - 1) // NCORES          # nodes per core
    shp = ((sh + 127) // 128) * 128          # padded to 128
    nw = shp // 128                          # dst windows per core
    # split each core's shard into NPASS quarters (tile-aligned to TPB) so
    # AllGather q can fire as soon as phase 1 finishes quarter q
    npass = 4
    base = (nw // npass) // TPB * TPB
    q_tiles = [base] * (npass - 1) + [nw - base * (npass - 1)]
    q_rows = [t * 128 for t in q_tiles]
    q_off = list(np.cumsum([0] + q_rows[:-1]))
    chunk_rows = [NCORES * r for r in q_rows]          # table rows per chunk
    assert max(chunk_rows) < 32768, chunk_rows
    return dict(N=N, F=F, C=C, SH=sh, SHP=shp, NW=nw, NPASS=npass,
                CHUNK_ROWS=chunk_rows, Q_TILES=q_tiles, Q_ROWS=q_rows,
                Q_OFF=q_off)


def _host_prep(in_feat, W, b, src, dst, cfg):
    """Shard + reformat inputs; build core-uniform edge-tile structure."""
    N, F, C = cfg["N"], cfg["F"], cfg["C"]
    SH, SHP, NW, NPASS = cfg["SH"], cfg["SHP"], cfg["NW"], cfg["NPASS"]

    src = np.asarray(src, dtype=np.int64)
    dst = np.asarray(dst, dtype=np.int64)

    # chunk q = quarter q of EVERY core's shard (so AllGather q only needs
    # phase-1 quarter q); within chunk q rows are core-major
    q_rows = np.asarray(cfg["Q_ROWS"], dtype=np.int64)
    q_off = np.asarray(cfg["Q_OFF"], dtype=np.int64)
    score = src // SH
    slocal = src % SH
    passno = (np.searchsorted(q_off, slocal, side="right") - 1).astype(np.int64)
    idx16 = (score * q_rows[passno] + (slocal - q_off[passno])).astype(np.int32)
    passno = passno.astype(np.int32)
    core = (dst // SH).astype(np.int32)
    dloc = (dst % SH).astype(np.int64)
    wno = (dloc // 128).astype(np.int32)
    drel = (dloc % 128).astype(np.int32)

    # per-(core, pass, window) counts -> uniform tile structure
    key = (core.astype(np.int64) * NPASS + passno) * NW + wno
    counts = np.bincount(key, minlength=NCORES * NPASS * NW).reshape(
        NCORES, NPASS, NW
    )
    t_pw = np.maximum((counts + 127) // 128, 1).max(axis=0)  # [NPASS, NW]

    # tile -> window mapping per pass, segment offsets
    seg_off = np.zeros((NPASS, NW), dtype=np.int64)  # tile offset of each segment
    pass_tiles = t_pw.sum(axis=1)                    # tiles per pass
    for p in range(NPASS):
        seg_off[p] = np.cumsum(t_pw[p]) - t_pw[p]
    ntiles = int(pass_tiles.sum())

    # per-core streams
    in_feat = np.asarray(in_feat, dtype=np.float32)
    WT = np.ascontiguousarray(np.asarray(W, dtype=np.float32).T)  # [F, C]
    bias = np.asarray(b, dtype=np.float32).reshape(C, 1)
    ident = np.eye(C, dtype=np.float32)

    deg_out = np.bincount(src, minlength=N)
    deg_in = np.bincount(dst, minlength=N)
    rp_out = np.concatenate([[0], np.cumsum(deg_out)])
    rp_in = np.concatenate([[0], np.cumsum(deg_in)])

    iota = np.tile(np.arange(128, dtype=np.float32), (128, 16)).astype(
        ml_dtypes.bfloat16
    )  # [128, 2048]: 16 copies of 0..127 per row

    order = np.lexsort((idx16, wno, passno, core))
    so_pass, so_idx, so_rel = passno[order], idx16[order], drel[order]
    so_core, so_w = core[order], wno[order]
    # edge range per (core, pass, window) in sorted order
    seg_start = np.searchsorted(
        (so_core.astype(np.int64) * NPASS + so_pass) * NW + so_w,
        np.arange(NCORES * NPASS * NW + 1),
    ).reshape(-1)

    TPB = 4
    NBLK = (NW + TPB - 1) // TPB
    in_maps = []
    for k in range(NCORES):
        inT = np.zeros((F, SHP), dtype=np.float32)
        lo, hi = k * SH, min((k + 1) * SH, N)
        inT[:, : hi - lo] = in_feat[lo:hi].T
        # [p, blk, c, n]: partition-contiguous per phase-1 block DMA
        inT_p = np.zeros((F, NBLK * TPB * 128), dtype=np.float32)
        inT_p[:, :SHP] = inT
        v = inT_p.reshape(F // 128, 128, NBLK, TPB * 128)
        inT_t = np.ascontiguousarray(v.transpose(1, 2, 0, 3))

        def rp_mats(rp):
            v = rp[lo : hi + 1]
            v = np.concatenate([v, np.full(SHP + 1 - len(v), v[-1], v.dtype)])
            lo_m = v[:SHP].reshape(NW, 128).T.astype(np.int32)
            hi_m = v[1 : SHP + 1].reshape(NW, 128).T.astype(np.int32)
            return np.ascontiguousarray(lo_m), np.ascontiguousarray(hi_m)

        rpo_lo, rpo_hi = rp_mats(rp_out)
        rpi_lo, rpi_hi = rp_mats(rp_in)

        idx_stream = np.zeros(ntiles * 128, dtype=np.int16)
        rel_stream = np.full(ntiles * 128, -1.0, dtype=np.float32)
        tile_base = 0
        for p in range(NPASS):
            for w in range(NW):
                s0 = seg_start[(k * NPASS + p) * NW + w]
                s1 = seg_start[(k * NPASS + p) * NW + w + 1]
                off = (tile_base + seg_off[p, w]) * 128
                idx_stream[off : off + (s1 - s0)] = so_idx[s0:s1]
                rel_stream[off : off + (s1 - s0)] = so_rel[s0:s1]
            tile_base += int(pass_tiles[p])

        idx_w = np.tile(
            np.ascontiguousarray(idx_stream.reshape(-1, 16).T), (8, 1)
        )  # [128, ntiles*8]: 16-part wrap replicated per Q7 core
        rel_m = np.ascontiguousarray(
            rel_stream.reshape(ntiles, 128).T.astype(ml_dtypes.bfloat16)
        )  # [128, ntiles]

        in_maps.append(
            {
                "inT": inT_t,
                "wt": WT,
                "bias": bias,
                "ident": ident,
                "iota": iota,
                "rpo_lo": rpo_lo,
                "rpo_hi": rpo_hi,
                "rpi_lo": rpi_lo,
                "rpi_hi": rpi_hi,
                "idx": idx_w,
                "rel": rel_m,
            }
        )

    struct = dict(t_pw=t_pw, pass_tiles=pass_tiles, ntiles=ntiles)
    return in_maps, struct


def _build_program(cfg, struct):
    F, C = cfg["F"], cfg["C"]
    SHP, NW, NPASS = cfg["SHP"], cfg["NW"], cfg["NPASS"]
    CHUNK_ROWS = cfg["CHUNK_ROWS"]
    t_pw, pass_tiles, ntiles = (
        struct["t_pw"],
        struct["pass_tiles"],
        struct["ntiles"],
    )
    KC = F // 128  # contraction chunks in phase 1
    F32R = mybir.dt.float32r

    nc = bacc.Bacc(
        "TRN2", target_bir_lowering=False, debug=False, num_devices=NCORES,
        num_swdge_queues=4,
    )

    NBLK = (NW + 3) // 4
    inT_d = nc.dram_tensor(
        "inT", [128, NBLK, F // 128, 4 * 128], F32R, kind="ExternalInput"
    ).ap()
    wt_d = nc.dram_tensor("wt", [F, C], F32R, kind="ExternalInput").ap()
    bias_d = nc.dram_tensor("bias", [C, 1], F32, kind="ExternalInput").ap()
    ident_d = nc.dram_tensor("ident", [C, C], F32, kind="ExternalInput").ap()
    iota_d = nc.dram_tensor("iota", [128, 2048], BF16, kind="ExternalInput").ap()
    rp_d = {
        n: nc.dram_tensor(n, [128, NW], I32, kind="ExternalInput").ap()
        for n in ("rpo_lo", "rpo_hi", "rpi_lo", "rpi_hi")
    }
    idx_d = nc.dram_tensor(
        "idx", [128, ntiles * 8], I16, kind="ExternalInput"
    ).ap()
    rel_d = nc.dram_tensor("rel", [128, ntiles], BF16, kind="ExternalInput").ap()
    hout_d = nc.dram_tensor("hout", [SHP, C], F32, kind="ExternalOutput").ap()

    with tile.TileContext(nc) as tc:
        with (
            tc.tile_pool(name="const", bufs=1) as cpool,
            tc.tile_pool(name="bigbuf", bufs=1) as bpool,
            tc.tile_pool(name="inT", bufs=3) as ipool,
            tc.tile_pool(name="gat", bufs=8) as gpool,
            tc.tile_pool(name="idxs", bufs=4) as idxpool,
            tc.tile_pool(name="sbuild", bufs=3) as spool,
            tc.tile_pool(name="blend", bufs=3) as blpool,
            tc.tile_pool(name="ps1", bufs=2, space="PSUM") as ps1,
            tc.tile_pool(name="pse", bufs=4, space="PSUM") as pse,
            tc.tile_pool(name="dram", bufs=1, space="DRAM") as dpool,
        ):
            # ---- load constants ----
            wt_s = cpool.tile([128, KC, C], F32R, tag="wt")
            nc.sync.dma_start(
                wt_s[:], wt_d.rearrange("(c p) f -> p c f", p=128)
            )
            bias_s = cpool.tile([C, 1], F32, tag="bias")
            nc.sync.dma_start(bias_s[:], bias_d)
            ident_s = cpool.tile([C, C], F32, tag="ident")
            nc.sync.dma_start(ident_s[:], ident_d)
            iota_s = cpool.tile([128, 2048], BF16, tag="iota")
            nc.sync.dma_start(iota_s[:], iota_d)
            rel_s = cpool.tile([128, ntiles], BF16, tag="rel")
            nc.sync.dma_start(rel_s[:], rel_d)
            rp_s = {}
            for n in rp_d:
                rp_s[n] = cpool.tile([128, NW], I32, tag=n, name=n)
                nc.sync.dma_start(rp_s[n][:], rp_d[n])

            # ---- degree norms: norm = sqrt(scale / clip(deg, 1)) ----
            def make_norm(lo, hi, scale, tag):
                deg = cpool.tile([128, NW], F32, tag=tag + "_deg")
                nc.vector.tensor_tensor(
                    deg[:], hi[:], lo[:], op=mybir.AluOpType.subtract
                )
                nc.vector.tensor_scalar_max(deg[:], deg[:], 1.0)
                rec = cpool.tile([128, NW], F32, tag=tag + "_rec")
                nc.vector.reciprocal(rec[:], deg[:])
                norm = cpool.tile([128, NW], F32, tag=tag)
                nc.scalar.activation(
                    norm[:], rec[:], mybir.ActivationFunctionType.Sqrt,
                    scale=scale,
                )
                return norm

            norm_out = make_norm(rp_s["rpo_lo"], rp_s["rpo_hi"], 1.0, "nout")
            norm_in = make_norm(
                rp_s["rpi_lo"], rp_s["rpi_hi"], (1.0 - ALPHA) ** 2, "nin"
            )

            # ---- big SBUF buffers ----
            h0s_s = bpool.tile([128, NW, 128], BF16, tag="h0s")  # padded table
            nc.vector.memset(h0s_s[:, :, C:128], 0.0)
            h0b_s = bpool.tile([128, NW, C], F32, tag="h0b")  # 0.2*h0, then out
            acc_s = bpool.tile([128, NW, C], F32, tag="acc")

            # ---- phase 1: psum h0^T = W @ inT (fp32r, moving dim =
            # nodes -> full PE rate), bias on the psum copy, PE-transpose
            # back to node-major. Quarter q feeds AllGather #q so edge
            # pass q overlaps later quarters. ----
            coll_ins = []
            h0s_dram = dpool.tile([SHP, 128], BF16, tag="h0s_dram",
                                  name="h0s_dram")
            tables = [
                dpool.tile([CHUNK_ROWS[q], 128], BF16, tag=f"table{q}",
                           name=f"table{q}", addr_space="Shared")
                for q in range(NPASS)
            ]
            SBATCH = 16  # S one-hots built per DVE op
            Q_TILES = cfg["Q_TILES"]
            qt_off = list(np.cumsum([0] + Q_TILES[:-1]))

            def emit_quarter_write(q):
                a, b = qt_off[q], qt_off[q] + Q_TILES[q]
                nc.sync.dma_start(
                    h0s_dram[a * 128:b * 128].rearrange(
                        "(t p) f -> p t f", p=128
                    ),
                    h0s_s[:, a:b, :],
                )

            def emit_quarter_coll(q):
                a, b = qt_off[q], qt_off[q] + Q_TILES[q]
                ci = nc.gpsimd.collective_compute(
                    "AllGather",
                    mybir.AluOpType.bypass,
                    replica_groups=[list(range(NCORES))],
                    ins=[h0s_dram[a * 128:b * 128].opt()],
                    outs=[tables[q][:].opt()],
                )
                coll_ins.append(ci.ins)

            def emit_quarter_gather(q):
                emit_quarter_write(q)
                if q == 0:
                    emit_quarter_coll(0)

            def emit_phase1():
                nq = 0
                for g0 in range(0, NW, TPB):
                    nb = min(TPB, NW - g0)
                    blk0 = g0
                    t = ipool.tile([128, KC, TPB * 128], F32R, tag="inT",
                                   name="t")
                    nc.sync.dma_start(t[:], inT_d[:, blk0 // TPB, :, :])
                    psT = ps1.tile([C, TPB * 128], F32, tag="psT", name="psT")
                    for c in range(KC):
                        nc.tensor.matmul(
                            psT[:, : nb * 128],
                            lhsT=wt_s[:, c, :],
                            rhs=t[:, c, : nb * 128],
                            start=(c == 0),
                            stop=(c == KC - 1),
                        )
                    h0T = ipool.tile([C, TPB * 128], F32, tag="h0T",
                                     name="h0T")
                    nc.vector.tensor_scalar(
                        h0T[:, : nb * 128], psT[:, : nb * 128], bias_s[:],
                        None, op0=mybir.AluOpType.add,
                    )
                    for j in range(nb):
                        tt = blk0 + j
                        pst = ps1.tile([128, C], F32, tag="pst", name="pst")
                        nc.tensor.transpose(
                            pst[:], h0T[:, j * 128 : (j + 1) * 128], ident_s[:]
                        )
                        nc.vector.tensor_scalar(
                            h0s_s[:, tt, 0:C], pst[:],
                            norm_out[:, tt : tt + 1],
                            None, op0=mybir.AluOpType.mult,
                        )
                        nc.scalar.activation(
                            h0b_s[:, tt, :], pst[:],
                            mybir.ActivationFunctionType.Copy, scale=ALPHA,
                        )
                    # quarter writes happen here; only coll 0 fires during
                    # phase 1 — the gpsimd engine is in-order, so later
                    # coll TRIGGERS (which wait on their quarter's write)
                    # must not queue ahead of pass-0's gathers. Colls 1..3
                    # are interleaved into pass 0's slab stream instead.
                    while nq < NPASS and g0 + nb >= qt_off[nq] + Q_TILES[nq]:
                        emit_quarter_gather(nq)
                        nq += 1
                assert nq == NPASS, (nq, NPASS)

            w_of_tile = [np.repeat(np.arange(NW), t_pw[p]) for p in range(NPASS)]
            seg_off_p = [np.cumsum(t_pw[p]) - t_pw[p] for p in range(NPASS)]
            st = dict(gtile=0, nslab=0, s_cur=None, it=None, it_off=None)

            def emit_pass(p):
                tbl = tables[p][:]
                ntp = int(pass_tiles[p])
                first_of_pass = True
                pos = 0
                nslab_in_pass = 0
                while pos < ntp:
                    if p == 0 and nslab_in_pass in (2, 4, 6):
                        # slip the next quarter's AllGather trigger between
                        # gathers so it issues as soon as its data is ready
                        q = nslab_in_pass // 2
                        if len(coll_ins) == q:
                            emit_quarter_coll(q)
                    nslab_in_pass += 1
                    nts = min(SLAB_TILES, ntp - pos)
                    gtile = st["gtile"]
                    g = gpool.tile([128, SLAB_TILES, 128], BF16, tag="gat",
                                   name="g")
                    nidx = nts * 128
                    IB = SLAB_TILES * 8  # idx cols per slab
                    if st["it_off"] is None or st["it_off"] + nidx // 16 > 2 * IB:
                        st["it"] = idxpool.tile([128, 2 * IB], I16,
                                                tag="idxs", name="it")
                        ncols = min(2 * IB, ntiles * 8 - gtile * 8)
                        nc.sync.dma_start(
                            st["it"][:, :ncols],
                            idx_d[:, gtile * 8 : gtile * 8 + ncols],
                        )
                        st["it_off"] = 0
                    it_lo = st["it_off"]
                    st["it_off"] += nidx // 16
                    gi = nc.gpsimd.dma_gather(
                        g[:, :nts, :],
                        tbl,
                        st["it"][:, it_lo : it_lo + nidx // 16],
                        num_idxs=nidx,
                        num_idxs_reg=nidx,
                        elem_size=128,
                        single_packet=False,
                        queue_num=st["nslab"] % 4,
                    )
                    st["nslab"] += 1
                    if first_of_pass:
                        # order this pass's first gather after its AllGather
                        tile.add_dep_helper(gi.ins, coll_ins[p], sync=True)
                        first_of_pass = False
                    for ti in range(nts):
                        if GATHER_ONLY:
                            break
                        tt = gtile + ti
                        if tt % SBATCH == 0:  # build S batch
                            st["s_cur"] = spool.tile(
                                [128, SBATCH * 128], BF16, tag="sb", name="s"
                            )
                            nb4 = min(SBATCH, ntiles - tt)
                            rel_b = rel_s[:, tt : tt + nb4].unsqueeze(-1)
                            sb_i = nc.vector.tensor_tensor(
                                st["s_cur"][:, : nb4 * 128].rearrange(
                                    "p (a b) -> p a b", b=128
                                ),
                                iota_s[:, : nb4 * 128].rearrange(
                                    "p (a b) -> p a b", b=128
                                ),
                                rel_b.broadcast_to((128, nb4, 128)),
                                op=mybir.AluOpType.is_equal,
                            )
                            if tt == 0:
                                # keep S-builds behind phase-1 DVE work
                                tile.add_dep_helper(
                                    sb_i.ins, coll_ins[0], sync=True
                                )
                        w = int(w_of_tile[p][pos + ti])
                        tloc = pos + ti - int(seg_off_p[p][w])
                        first = tloc == 0
                        last = tloc == t_pw[p][w] - 1
                        if first:
                            st["ps"] = pse.tile([128, C], F32, tag="pse",
                                                name="ps")
                        cur_ps = st["ps"]
                        sc = tt % SBATCH
                        nc.tensor.matmul(
                            cur_ps[:],
                            lhsT=st["s_cur"][:, sc * 128 : (sc + 1) * 128],
                            rhs=g[:, ti, 0:C],
                            start=first,
                            stop=last,
                        )
                        if last:
                            if p == 0:
                                nc.vector.tensor_copy(
                                    acc_s[:, w, :], cur_ps[:]
                                )
                            else:
                                nc.vector.tensor_tensor(
                                    acc_s[:, w, :], cur_ps[:], acc_s[:, w, :],
                                    op=mybir.AluOpType.add,
                                )
                    st["gtile"] += nts
                    pos += nts

            if SKIP_EDGES:
                pass_tiles = [0] * NPASS
            emit_phase1()
            for p in range(NPASS):
                if p > 0:
                    while len(coll_ins) <= p:  # pass 0 too short to slip all
                        emit_quarter_coll(len(coll_ins))
                emit_pass(p)

            # ---- blend + store ----
            for w in range(NW):
                if SKIP_EDGES or GATHER_ONLY:
                    break
                t1 = blpool.tile([128, C], F32, tag="t1")
                nc.vector.tensor_scalar(
                    t1[:], acc_s[:, w, :], norm_in[:, w : w + 1], None,
                    op0=mybir.AluOpType.mult,
                )
                nc.vector.tensor_tensor(
                    h0b_s[:, w, :], t1[:], h0b_s[:, w, :],
                    op=mybir.AluOpType.add,
                )
            nc.sync.dma_start(
                hout_d.rearrange("(t p) f -> p t f", p=128), h0b_s[:]
            )

    nc.compile()
    return nc


_RESULT_CACHE = {}


def run(in_feat, W, b, src, dst, trace=False):
    N, F = in_feat.shape
    C = W.shape[0]
    cfg = _cfg(N, F, C)
    in_maps, struct = _host_prep(in_feat, W, b, src, dst, cfg)
    nc = _build_program(cfg, struct)
    res = run_bass_kernel_spmd(
        nc, in_maps, list(range(NCORES)), trace=trace
    )
    outs = [res.results[k]["hout"][: cfg["SH"]] for k in range(NCORES)]
    full = np.concatenate(outs, axis=0)[:N].astype(np.float32)
    return full, res


def kernel(in_feat, W, b, src, dst):
    full, _ = run(in_feat, W, b, src, dst)
    return full

